# Initial kernel scaffold
#
"""Trainium2 Bass kernel for nn_NetStackedHourglass_2 keypoint reduction.

Full inputs in, full output out. Internally: pure data-parallel across 8
NeuronCores (32 batches each). Each core computes per-(batch,channel) masked
spatial reductions; the tiny [B,20,*] -> [B,21,2] keypoint assembly runs on
host.
"""

import sys

if "/opt/trn_rl_repo" not in sys.path:
    sys.path.insert(0, "/opt/trn_rl_repo")

import numpy as np

import concourse.bass as bass
import concourse.tile as tile
from concourse import mybir
from concourse.bass_utils import run_bass_kernel_spmd

N_CORES = 8
B_FULL = 256
B_SHARD = B_FULL // N_CORES  # 32
C = 20
RES = 64
SPATIAL = RES * RES          # 4096
ROWS = B_SHARD * C           # 640 (b,c) rows per core
P = 128                      # partitions
GROUPS = ROWS // P           # 5
CHUNK = 2048                 # spatial elements per tile
NCHUNK = SPATIAL // CHUNK    # 2
EPS = 1e-6

F32 = mybir.dt.float32


def _build_program() -> bass.Bass:
    nc = bass.Bass()

    fv = nc.declare_dram_parameter("front_vec", [ROWS, SPATIAL, 2], F32, isOutput=False)
    fd = nc.declare_dram_parameter("front_dis", [ROWS, SPATIAL], F32, isOutput=False)
    bv = nc.declare_dram_parameter("back_vec", [ROWS, SPATIAL, 2], F32, isOutput=False)
    bd = nc.declare_dram_parameter("back_dis", [ROWS, SPATIAL], F32, isOutput=False)
    m = nc.declare_dram_parameter("ske_mask", [ROWS, SPATIAL], F32, isOutput=False)
    # stats columns: 0:Fx 1:Fy 2:Bx 3:By 4:msum 5-7:pad(garbage)
    stats = nc.declare_dram_parameter("stats", [ROWS, 8], F32, isOutput=True)

    MULT = mybir.AluOpType.mult
    ADD = mybir.AluOpType.add

    with tile.TileContext(nc) as tc:
        with (
            tc.tile_pool(name="singles", bufs=1) as singles,
            tc.tile_pool(name="io_small", bufs=2) as io_small,
            tc.tile_pool(name="io_big", bufs=2) as io_big,
            tc.tile_pool(name="work", bufs=1) as work,
            tc.tile_pool(name="acc", bufs=2) as accp,
            tc.tile_pool(name="psum", bufs=2, space="PSUM") as psum,
        ):
            # --- constants: loc_x per chunk (p//64 within full spatial), loc_y ---
            loci = singles.tile([P, CHUNK], mybir.dt.int32)
            locx = []
            for ci in range(NCHUNK):
                t = singles.tile([P, CHUNK], F32, tag=f"locx{ci}")
                # values: base + i0 for layout (rows of 64): p//64
                nc.gpsimd.iota(
                    loci,
                    pattern=[[1, CHUNK // RES], [0, RES]],
                    base=ci * (CHUNK // RES),
                    channel_multiplier=0,
                )
                nc.vector.tensor_copy(out=t, in_=loci)
                locx.append(t)
            locy = singles.tile([P, CHUNK], F32)
            nc.gpsimd.iota(
                loci,
                pattern=[[0, CHUNK // RES], [1, RES]],
                base=0,
                channel_multiplier=0,
            )
            nc.vector.tensor_copy(out=locy, in_=loci)

            for g in range(GROUPS):
                r0 = g * P
                # accumulators: [chunk, quantity]; quantities:
                # 0:fx 1:fy 2:bx 3:by 4:lx 5:ly 6:msum
                acc_t = accp.tile([P, NCHUNK, 8], F32, tag="acc")
                for ci in range(NCHUNK):
                    c0 = ci * CHUNK
                    m_t = io_small.tile([P, CHUNK], F32, tag="m")
                    fd_t = io_small.tile([P, CHUNK], F32, tag="fd")
                    bd_t = io_small.tile([P, CHUNK], F32, tag="bd")
                    fv_t = io_big.tile([P, CHUNK, 2], F32, tag="fv")
                    bv_t = io_big.tile([P, CHUNK, 2], F32, tag="bv")

                    nc.sync.dma_start(out=m_t, in_=m[r0 : r0 + P, c0 : c0 + CHUNK])
                    nc.sync.dma_start(out=fd_t, in_=fd[r0 : r0 + P, c0 : c0 + CHUNK])
                    nc.sync.dma_start(out=bd_t, in_=bd[r0 : r0 + P, c0 : c0 + CHUNK])
                    nc.sync.dma_start(out=fv_t, in_=fv[r0 : r0 + P, c0 : c0 + CHUNK, :])
                    nc.sync.dma_start(out=bv_t, in_=bv[r0 : r0 + P, c0 : c0 + CHUNK, :])

                    t_f = work.tile([P, CHUNK], F32, tag="t_f")
                    t_b = work.tile([P, CHUNK], F32, tag="t_b")
                    scr = work.tile([P, CHUNK], F32, tag="scr")
                    pscr = psum.tile([P, CHUNK], F32, tag="pscr")

                    # msum on ScalarE (activation w/ accumulate; out -> PSUM)
                    nc.scalar.activation(
                        out=pscr,
                        in_=m_t,
                        func=mybir.ActivationFunctionType.Copy,
                        accum_out=acc_t[:, ci, 6:7],
                    )

                    nc.vector.tensor_mul(t_f, m_t, fd_t)
                    nc.vector.tensor_mul(t_b, m_t, bd_t)

                    def ttr(in0, in1, out, qi, scale):
                        nc.vector.tensor_tensor_reduce(
                            out=out,
                            in0=in0,
                            in1=in1,
                            scale=scale,
                            scalar=0.0,
                            op0=MULT,
                            op1=ADD,
                            accum_out=acc_t[:, ci, qi : qi + 1],
                        )

                    # front/back votes: sum(t * vec) * 64  (scale folds res=64)
                    ttr(fv_t[:, :, 0], t_f, fd_t, 0, 64.0)  # fd_t dead -> reuse
                    ttr(fv_t[:, :, 1], t_f, t_f, 1, 64.0)   # last t_f use, in-place
                    ttr(bv_t[:, :, 0], t_b, bd_t, 2, 64.0)  # bd_t dead -> reuse
                    ttr(bv_t[:, :, 1], t_b, t_b, 3, 64.0)   # last t_b use, in-place
                    # mask-location moments
                    ttr(m_t, locx[ci], scr, 4, 1.0)
                    ttr(m_t, locy, m_t, 5, 1.0)             # last m_t use, in-place

                # --- combine chunks + finalize per-partition stats ---
                sum_t = accp.tile([P, 8], F32, tag="sum")
                stats_t = accp.tile([P, 8], F32, tag="stats")
                nc.vector.tensor_add(sum_t, acc_t[:, 0, :], acc_t[:, 1, :])
                denom = accp.tile([P, 1], F32, tag="denom")
                recip = accp.tile([P, 1], F32, tag="recip")
                nc.vector.tensor_scalar_add(denom, sum_t[:, 6:7], EPS)
                nc.vector.reciprocal(recip, denom)
                # F/B = (dot + loc_moment) / denom
                for qi, li, oi in ((0, 4, 0), (1, 5, 1), (2, 4, 2), (3, 5, 3)):
                    nc.vector.tensor_scalar(
                        out=stats_t[:, oi : oi + 1],
                        in0=sum_t[:, qi : qi + 1],
                        scalar1=sum_t[:, li : li + 1],
                        scalar2=recip,
                        op0=ADD,
                        op1=MULT,
                    )
                nc.vector.tensor_copy(out=stats_t[:, 4:8], in_=sum_t[:, 4:8])
                nc.sync.dma_start(out=stats[r0 : r0 + P, :], in_=stats_t)

    return nc


_PROGRAM_CACHE: dict = {}


def _get_program() -> bass.Bass:
    if "nc" not in _PROGRAM_CACHE:
        _PROGRAM_CACHE["nc"] = _build_program()
    return _PROGRAM_CACHE["nc"]


def _run_device(in_maps, trace=False, **kwargs):
    nc = _get_program()
    return run_bass_kernel_spmd(nc, in_maps, list(range(N_CORES)), trace=trace, **kwargs)


def _make_in_maps(front_vec, front_dis, back_vec, back_dis, ske_mask):
    fv = np.ascontiguousarray(np.asarray(front_vec, dtype=np.float32))
    fd = np.ascontiguousarray(np.asarray(front_dis, dtype=np.float32))
    bv = np.ascontiguousarray(np.asarray(back_vec, dtype=np.float32))
    bd = np.ascontiguousarray(np.asarray(back_dis, dtype=np.float32))
    m = np.ascontiguousarray(np.asarray(ske_mask, dtype=np.float32))
    in_maps = []
    for i in range(N_CORES):
        sl = slice(i * B_SHARD, (i + 1) * B_SHARD)
        in_maps.append(
            {
                "front_vec": fv[sl].reshape(ROWS, SPATIAL, 2),
                "front_dis": fd[sl].reshape(ROWS, SPATIAL),
                "back_vec": bv[sl].reshape(ROWS, SPATIAL, 2),
                "back_dis": bd[sl].reshape(ROWS, SPATIAL),
                "ske_mask": m[sl].reshape(ROWS, SPATIAL),
            }
        )
    return in_maps


def _assemble(stats: np.ndarray) -> np.ndarray:
    """stats: [B, 20, >=5] with cols Fx Fy Bx By msum -> kp [B, 21, 2]."""
    B = stats.shape[0]
    F_ = stats[:, :, 0:2].astype(np.float32)
    Bk = stats[:, :, 2:4].astype(np.float32)
    msum = stats[:, :, 4].astype(np.float32)

    root_terms = np.where(
        (msum[:, ::4] != 0.0)[..., None], Bk[:, ::4], np.float32(0.0)
    )  # [B,5,2]
    kp0 = root_terms.sum(axis=1, dtype=np.float32) / np.float32(5.0)  # [B,2]

    Fg = F_.reshape(B, 5, 4, 2)
    Bg = Bk.reshape(B, 5, 4, 2)
    tail = np.stack(
        [
            Fg[:, :, 3],
            (Fg[:, :, 2] + Bg[:, :, 3]) * np.float32(0.5),
            (Fg[:, :, 1] + Bg[:, :, 2]) * np.float32(0.5),
            (Fg[:, :, 0] + Bg[:, :, 1]) * np.float32(0.5),
        ],
        axis=2,
    )  # [B,5,4,2]
    kp = np.concatenate([kp0[:, None], tail.reshape(B, 20, 2)], axis=1)
    return (kp * np.float32(4.0)).astype(np.float32)


def kernel(front_vec, front_dis, back_vec, back_dis, ske_mask) -> np.ndarray:
    in_maps = _make_in_maps(front_vec, front_dis, back_vec, back_dis, ske_mask)
    res = _run_device(in_maps)
    stats = np.stack([np.asarray(res.results[i]["stats"]) for i in range(N_CORES)])
    stats = stats.reshape(B_FULL, C, 8)
    return _assemble(stats)


# revision 10
# speedup vs baseline: 1.1802x; 1.1802x over previous
"""Trainium2 Bass kernel for nn_NetStackedHourglass_2 keypoint reduction.

Full inputs in, full output out. Internally: pure data-parallel across 8
NeuronCores (32 batches each). Each core computes per-(batch,channel) masked
spatial reductions; the tiny [B,20,*] -> [B,21,2] keypoint assembly runs on
host.
"""

import sys

if "/opt/trn_rl_repo" not in sys.path:
    sys.path.insert(0, "/opt/trn_rl_repo")

import numpy as np

import concourse.bass as bass
import concourse.tile as tile
from concourse import mybir
from concourse.bass_utils import run_bass_kernel_spmd

N_CORES = 8
B_FULL = 256
B_SHARD = B_FULL // N_CORES  # 32
C = 20
RES = 64
SPATIAL = RES * RES          # 4096
ROWS = B_SHARD * C           # 640 (b,c) rows per core
P = 128                      # partitions
GROUPS = ROWS // P           # 5
CHUNK = 2048                 # spatial elements per tile
NCHUNK = SPATIAL // CHUNK    # 2
EPS = 1e-6

F32 = mybir.dt.float32


def _build_program(repeat: int = 1) -> bass.Bass:
    nc = bass.Bass()

    fv = nc.declare_dram_parameter("front_vec", [ROWS, SPATIAL, 2], F32, isOutput=False)
    fd = nc.declare_dram_parameter("front_dis", [ROWS, SPATIAL], F32, isOutput=False)
    bv = nc.declare_dram_parameter("back_vec", [ROWS, SPATIAL, 2], F32, isOutput=False)
    bd = nc.declare_dram_parameter("back_dis", [ROWS, SPATIAL], F32, isOutput=False)
    m = nc.declare_dram_parameter("ske_mask", [ROWS, SPATIAL], F32, isOutput=False)
    # rows: locx per chunk (NCHUNK rows), then locy
    loc = nc.declare_dram_parameter("loc_const", [NCHUNK + 1, CHUNK], F32, isOutput=False)
    # stats columns: 0:Fx 1:Fy 2:Bx 3:By 4:msum 5-7:pad(garbage)
    stats = nc.declare_dram_parameter("stats", [ROWS, 8], F32, isOutput=True)

    MULT = mybir.AluOpType.mult
    ADD = mybir.AluOpType.add

    with tile.TileContext(nc) as tc:
        with (
            tc.tile_pool(name="singles", bufs=1) as singles,
            tc.tile_pool(name="io_small", bufs=2) as io_small,
            tc.tile_pool(name="io_big", bufs=2) as io_big,
            tc.tile_pool(name="work", bufs=1) as work,
            tc.tile_pool(name="acc", bufs=2) as accp,
            tc.tile_pool(name="psum", bufs=2, space="PSUM") as psum,
        ):
            # --- constants: loc_x per chunk (p//64 within full spatial), loc_y ---
            locx = []
            for ci in range(NCHUNK):
                t = singles.tile([P, CHUNK], F32, tag=f"locx{ci}")
                nc.gpsimd.dma_start(
                    out=t, in_=loc[ci : ci + 1, :].to_broadcast([P, CHUNK])
                )
                locx.append(t)
            locy = singles.tile([P, CHUNK], F32)
            nc.gpsimd.dma_start(
                out=locy, in_=loc[NCHUNK : NCHUNK + 1, :].to_broadcast([P, CHUNK])
            )

            for g in range(GROUPS * repeat):
                g = g % GROUPS
                r0 = g * P
                # accumulators: [chunk, quantity]; quantities:
                # 0:fx 1:fy 2:bx 3:by 4:lx 5:ly 6:msum
                acc_t = accp.tile([P, NCHUNK, 8], F32, tag="acc")
                for ci in range(NCHUNK):
                    c0 = ci * CHUNK
                    m_t = io_small.tile([P, CHUNK], F32, tag="m")
                    fd_t = io_small.tile([P, CHUNK], F32, tag="fd")
                    bd_t = io_small.tile([P, CHUNK], F32, tag="bd")
                    fv_t = io_big.tile([P, CHUNK, 2], F32, tag="fv")
                    bv_t = io_big.tile([P, CHUNK, 2], F32, tag="bv")

                    nc.sync.dma_start(out=m_t, in_=m[r0 : r0 + P, c0 : c0 + CHUNK])
                    nc.sync.dma_start(out=fd_t, in_=fd[r0 : r0 + P, c0 : c0 + CHUNK])
                    nc.sync.dma_start(out=bd_t, in_=bd[r0 : r0 + P, c0 : c0 + CHUNK])
                    nc.sync.dma_start(out=fv_t, in_=fv[r0 : r0 + P, c0 : c0 + CHUNK, :])
                    nc.sync.dma_start(out=bv_t, in_=bv[r0 : r0 + P, c0 : c0 + CHUNK, :])

                    t_f = work.tile([P, CHUNK], F32, tag="t_f")
                    t_b = work.tile([P, CHUNK], F32, tag="t_b")
                    scr = work.tile([P, CHUNK], F32, tag="scr")
                    pscr = psum.tile([P, CHUNK], F32, tag="pscr")

                    # msum on ScalarE (activation w/ accumulate; out -> PSUM)
                    nc.scalar.activation(
                        out=pscr,
                        in_=m_t,
                        func=mybir.ActivationFunctionType.Copy,
                        accum_out=acc_t[:, ci, 6:7],
                    )

                    nc.vector.tensor_mul(t_f, m_t, fd_t)
                    nc.vector.tensor_mul(t_b, m_t, bd_t)

                    def fused_dot(in0, in1, out, qi, scale):
                        # out = (in0 * scale) * in1 ; accum = sum(out)
                        nc.vector.scalar_tensor_tensor(
                            out=out,
                            in0=in0,
                            scalar=scale,
                            in1=in1,
                            op0=MULT,
                            op1=MULT,
                            accum_out=acc_t[:, ci, qi : qi + 1],
                        )

                    # front/back votes: sum(t * vec) * 64  (scalar folds res=64)
                    fused_dot(fv_t[:, :, 0], t_f, fd_t, 0, 64.0)  # fd_t dead -> reuse
                    fused_dot(fv_t[:, :, 1], t_f, t_f, 1, 64.0)   # last t_f use
                    fused_dot(bv_t[:, :, 0], t_b, bd_t, 2, 64.0)  # bd_t dead -> reuse
                    fused_dot(bv_t[:, :, 1], t_b, t_b, 3, 64.0)   # last t_b use
                    # mask-location moments
                    fused_dot(m_t, locx[ci], scr, 4, 1.0)
                    fused_dot(m_t, locy, m_t, 5, 1.0)             # last m_t use

                # --- combine chunks + finalize per-partition stats ---
                sum_t = accp.tile([P, 8], F32, tag="sum")
                stats_t = accp.tile([P, 8], F32, tag="stats")
                nc.vector.tensor_add(sum_t, acc_t[:, 0, :], acc_t[:, 1, :])
                denom = accp.tile([P, 1], F32, tag="denom")
                recip = accp.tile([P, 1], F32, tag="recip")
                nc.vector.tensor_scalar_add(denom, sum_t[:, 6:7], EPS)
                nc.vector.reciprocal(recip, denom)
                # F/B = (dot + loc_moment) / denom
                for qi, li, oi in ((0, 4, 0), (1, 5, 1), (2, 4, 2), (3, 5, 3)):
                    nc.vector.tensor_scalar(
                        out=stats_t[:, oi : oi + 1],
                        in0=sum_t[:, qi : qi + 1],
                        scalar1=sum_t[:, li : li + 1],
                        scalar2=recip,
                        op0=ADD,
                        op1=MULT,
                    )
                nc.vector.tensor_copy(out=stats_t[:, 4:8], in_=sum_t[:, 4:8])
                nc.sync.dma_start(out=stats[r0 : r0 + P, :], in_=stats_t)

    from concourse.library_overlay import lower_extended_insts

    lower_extended_insts(nc)
    _legalize_waits(nc)
    return nc


def _legalize_waits(nc) -> None:
    """walrus codegen allows 1 sync-wait per instruction (2 for
    EventSemaphore). Hoist excess waits onto EventSemaphore carriers
    inserted just before the offending instruction on the same engine."""
    n_fix = 0
    for f in nc.m.functions:
        for blk in f.blocks:
            insts = blk.instructions
            new_list = []
            changed = False
            for ins in insts:
                si = getattr(ins, "sync_info", None)
                ow = list(si.on_wait) if (si is not None and si.on_wait) else []
                cap = 2 if isinstance(ins, mybir.InstEventSemaphore) else 1
                if len(ow) > cap:
                    excess, keep = ow[:-cap], ow[-cap:]
                    for j in range(0, len(excess), 2):
                        ev = mybir.InstEventSemaphore(
                            name=f"{ins.name}-lw{j}", ins=[], outs=[]
                        )
                        ev.engine = ins.engine
                        ev.sync_info = mybir.SyncInfo(
                            on_wait=excess[j : j + 2], on_update=[]
                        )
                        new_list.append(ev)
                    ins.sync_info = mybir.SyncInfo(
                        on_wait=keep,
                        on_update=list(si.on_update) if si.on_update else [],
                    )
                    changed = True
                    n_fix += 1
                new_list.append(ins)
            if changed:
                blk.instructions.clear()
                blk.instructions.extend(new_list)


_PROGRAM_CACHE: dict = {}


def _get_program() -> bass.Bass:
    if "nc" not in _PROGRAM_CACHE:
        _PROGRAM_CACHE["nc"] = _build_program()
    return _PROGRAM_CACHE["nc"]


def _run_device(in_maps, trace=False, **kwargs):
    nc = _get_program()
    return run_bass_kernel_spmd(nc, in_maps, list(range(N_CORES)), trace=trace, **kwargs)


def _make_in_maps(front_vec, front_dis, back_vec, back_dis, ske_mask):
    fv = np.ascontiguousarray(np.asarray(front_vec, dtype=np.float32))
    fd = np.ascontiguousarray(np.asarray(front_dis, dtype=np.float32))
    bv = np.ascontiguousarray(np.asarray(back_vec, dtype=np.float32))
    bd = np.ascontiguousarray(np.asarray(back_dis, dtype=np.float32))
    m = np.ascontiguousarray(np.asarray(ske_mask, dtype=np.float32))
    p = np.arange(SPATIAL)
    locx_full = (p // RES).astype(np.float32).reshape(NCHUNK, CHUNK)
    locy_row = (p[:CHUNK] % RES).astype(np.float32)
    loc_const = np.ascontiguousarray(
        np.concatenate([locx_full, locy_row[None, :]], axis=0)
    )
    in_maps = []
    for i in range(N_CORES):
        sl = slice(i * B_SHARD, (i + 1) * B_SHARD)
        in_maps.append(
            {
                "front_vec": fv[sl].reshape(ROWS, SPATIAL, 2),
                "front_dis": fd[sl].reshape(ROWS, SPATIAL),
                "back_vec": bv[sl].reshape(ROWS, SPATIAL, 2),
                "back_dis": bd[sl].reshape(ROWS, SPATIAL),
                "ske_mask": m[sl].reshape(ROWS, SPATIAL),
                "loc_const": loc_const,
            }
        )
    return in_maps


def _assemble(stats: np.ndarray) -> np.ndarray:
    """stats: [B, 20, >=5] with cols Fx Fy Bx By msum -> kp [B, 21, 2]."""
    B = stats.shape[0]
    F_ = stats[:, :, 0:2].astype(np.float32)
    Bk = stats[:, :, 2:4].astype(np.float32)
    msum = stats[:, :, 4].astype(np.float32)

    root_terms = np.where(
        (msum[:, ::4] != 0.0)[..., None], Bk[:, ::4], np.float32(0.0)
    )  # [B,5,2]
    kp0 = root_terms.sum(axis=1, dtype=np.float32) / np.float32(5.0)  # [B,2]

    Fg = F_.reshape(B, 5, 4, 2)
    Bg = Bk.reshape(B, 5, 4, 2)
    tail = np.stack(
        [
            Fg[:, :, 3],
            (Fg[:, :, 2] + Bg[:, :, 3]) * np.float32(0.5),
            (Fg[:, :, 1] + Bg[:, :, 2]) * np.float32(0.5),
            (Fg[:, :, 0] + Bg[:, :, 1]) * np.float32(0.5),
        ],
        axis=2,
    )  # [B,5,4,2]
    kp = np.concatenate([kp0[:, None], tail.reshape(B, 20, 2)], axis=1)
    return (kp * np.float32(4.0)).astype(np.float32)


def kernel(front_vec, front_dis, back_vec, back_dis, ske_mask) -> np.ndarray:
    in_maps = _make_in_maps(front_vec, front_dis, back_vec, back_dis, ske_mask)
    res = _run_device(in_maps)
    stats = np.stack([np.asarray(res.results[i]["stats"]) for i in range(N_CORES)])
    stats = stats.reshape(B_FULL, C, 8)
    return _assemble(stats)
